# revision 13
# baseline (speedup 1.0000x reference)
"""MoChA (monotonic chunkwise attention) Trainium2 Bass kernel.

Full inputs in, full outputs out. Data-parallel over batch: B=16 -> 2
samples on each of 8 NeuronCores. Per core, per sample:

  phase A: PE transposes of key/value/query; projections (fp32r matmuls,
           N>=512 -> full rate); monotonic energies -> 1-p = sigmoid(-e')
           -> lg = Ln(clip) -> exclusive cumsum (tensor_tensor_scan) ->
           cp = Exp, rcp = 1/clip(cp); G = pc_{q-1}*rcp_q built K-major
           via PE transposes; chunk energies -> softmax numerator sexp,
           moving-sum denominator, u = p*cp*rden spilled K-major to DRAM.
  phase B: alpha recurrence S_q = cumsum_k(g_q * S_{q-1}) as a (q, chunk)
           anti-diagonal wavefront: DVE mult + L-matmul (cumsum across
           partitions) + row-127-selector matmul (carry broadcast) + copy
           per diagonal. G and S share one SBUF buffer (each G cell is
           read exactly once, at the step that writes its S cell).
  phase C: beta^T = sexp^T * movsum8(alpha*rden)^T via banded matmuls
           K-major; cv accumulated on PE; output projection.

Self-contained: shapes hardcoded from the problem spec.
"""
import os
os.environ.setdefault("JAX_PLATFORMS", "axon")
from contextlib import ExitStack

import numpy as np

import concourse.bass as bass
import concourse.tile as tile
from concourse import bacc, mybir
from concourse.bass_utils import run_bass_kernel_spmd
from concourse.masks import make_identity

F32 = mybir.dt.float32
F32R = mybir.dt.float32r
AX = mybir.AxisListType
ALU = mybir.AluOpType
ACTF = mybir.ActivationFunctionType

B, Q, K, D = 16, 128, 1500, 512
ADIM = 512
H = 4
NCORE = 8
BL = B // NCORE
NCH = 12
P = 128
KP = NCH * P
NPAIR = BL * H
INV_S = float(1.0 / np.sqrt(np.float32(D)))
NB3 = (512, 512, 476)
NB3P = (512, 512, 512)

_CACHE = {}


def _aff(nc, t, compare_op, fill, base, step, cm=1):
    nc.gpsimd.affine_select(out=t, in_=t, compare_op=compare_op, fill=fill,
                            base=base, pattern=[[step, P]], channel_multiplier=cm)


def _emit(nc, tc, ctx, io):
    key_d, query_d, value_d, out_d = (io["key2"], io["query2"], io["value2"],
                                      io["out2"])
    u_d, v_d, sexpT_d = io["u_sc"], io["v_sc"], io["sexpT_sc"]
    dma = nc.default_dma_engine

    consts = ctx.enter_context(tc.tile_pool(name="consts", bufs=1))
    wpool = ctx.enter_context(tc.tile_pool(name="wpool", bufs=1))
    qpool = ctx.enter_context(tc.tile_pool(name="qpool", bufs=1))
    ktpool = ctx.enter_context(tc.tile_pool(name="ktpool", bufs=1))
    kcpool = ctx.enter_context(tc.tile_pool(name="kcpool", bufs=1))
    pairp = ctx.enter_context(tc.tile_pool(name="pairp", bufs=1))
    gsp = ctx.enter_context(tc.tile_pool(name="gsp", bufs=1))
    rdtp = ctx.enter_context(tc.tile_pool(name="rdtp", bufs=1))
    smallp = ctx.enter_context(tc.tile_pool(name="smallp", bufs=3))
    cpool = ctx.enter_context(tc.tile_pool(name="cpool", bufs=3))
    zpool = ctx.enter_context(tc.tile_pool(name="zpool", bufs=3))
    tpool = ctx.enter_context(tc.tile_pool(name="tpool", bufs=9))

    ps_tr = ctx.enter_context(tc.tile_pool(name="ps_tr", bufs=2, space="PSUM"))
    ps_mid = ctx.enter_context(tc.tile_pool(name="ps_mid", bufs=2, space="PSUM"))
    ps_ms = ctx.enter_context(tc.tile_pool(name="ps_ms", bufs=3, space="PSUM"))
    ps_sc = ctx.enter_context(tc.tile_pool(name="ps_sc", bufs=1, space="PSUM"))

    # ---------------- constants ----------------
    ident = consts.tile([P, P], F32, tag="ident")
    make_identity(nc, ident)
    ident_r = consts.tile([P, P], F32R, tag="ident_r")
    nc.vector.tensor_copy(ident_r, ident)

    L = consts.tile([P, P], F32, tag="L")          # L[p, m] = 1 if p <= m
    nc.gpsimd.memset(L, 0.0)
    _aff(nc, L, ALU.is_gt, 1.0, 0, -1)
    M127 = consts.tile([P, P], F32, tag="M127")    # row 127 = ones
    nc.gpsimd.memset(M127, 0.0)
    _aff(nc, M127, ALU.not_equal, 1.0, -127, 0)
    Band0 = consts.tile([P, P], F32, tag="Band0")  # 1 if 0 <= j - p <= 7
    nc.gpsimd.memset(Band0, 1.0)
    _aff(nc, Band0, ALU.is_gt, 0.0, 1, -1, cm=1)    # keep j-p+1 > 0
    _aff(nc, Band0, ALU.is_gt, 0.0, 8, 1, cm=-1)    # keep 8+p-j > 0
    Band1 = consts.tile([P, P], F32, tag="Band1")  # 1 if 1 <= j+128-p <= 7
    nc.gpsimd.memset(Band1, 1.0)
    _aff(nc, Band1, ALU.is_gt, 0.0, 128, -1, cm=1)  # keep j-p+128 > 0
    _aff(nc, Band1, ALU.is_gt, 0.0, -120, 1, cm=-1) # keep p-j-120 > 0

    ones_f = consts.tile([1, D], F32, tag="ones_f")
    nc.vector.memset(ones_f, 1.0)
    ones_r = consts.tile([1, D], F32R, tag="ones_r")
    nc.vector.tensor_copy(ones_r, ones_f)
    zeros_w = consts.tile([P, KP], F32, tag="zeros_w")
    nc.vector.memset(zeros_w, 0.0)

    r_t = consts.tile([P, 1], F32, tag="r_t")
    dma.dma_start(r_t, bass.AP(tensor=io["r"], offset=0, ap=[[0, P], [1, 1]]))
    neg_r = consts.tile([P, 1], F32, tag="neg_r")
    nc.vector.tensor_scalar_mul(neg_r, r_t, -1.0)

    def bias_tile(name):
        t = consts.tile([P, 4], F32, tag=name)
        dma.dma_start(t, bass.AP(tensor=io[name], offset=0, ap=[[1, P], [P, 4]]))
        return t

    bkm_t = bias_tile("bk_m")
    bqm_t = bias_tile("bq_m")
    bkc_t = bias_tile("bk_c")
    bqc_t = bias_tile("bq_c")
    bv_row = consts.tile([1, D], F32R, tag="bv_row")
    dma.dma_start(bv_row, io["bv"][:].unsqueeze(0))
    bout_row = consts.tile([1, D], F32R, tag="bout_row")
    dma.dma_start(bout_row, io["bout"][:].unsqueeze(0))

    def load_w(name):
        t = wpool.tile([P, 4, D], F32R, tag="w")
        dma.dma_start(t, io[name][:].rearrange("(t p) d -> p t d", p=P))
        return t

    def wsl(w, t, m):
        return w[:, t, m * P:(m + 1) * P]

    # ---------------- queryT / qmT / qcT (both samples upfront) ----------
    queryT = kcpool.tile([P, KP], F32R, tag="kc1")
    qT = queryT[:, 0:4 * BL * P].rearrange("p (t b q) -> p t b q", t=4, b=BL)
    for b in range(BL):
        qtile = smallp.tile([P, D], F32R, tag="ld")
        dma.dma_start(qtile, query_d[b])
        for t in range(4):
            ps = ps_tr.tile([P, P], F32R, tag="tr")
            nc.tensor.transpose(ps, qtile[:, t * P:(t + 1) * P], ident_r)
            nc.vector.tensor_copy(qT[:, t, b], ps)

    qmT = qpool.tile([P, 4, BL, P], F32R, tag="qmT")
    qcT = qpool.tile([P, 4, BL, P], F32R, tag="qcT")
    for wname, bt, dst in (("Wq_m", bqm_t, qmT), ("Wq_c", bqc_t, qcT)):
        w = load_w(wname)
        for m in range(4):
            ps = ps_mid.tile([P, 512], F32, tag="mid")
            for t in range(4):
                nc.tensor.matmul(ps[:, 0:BL * P], wsl(w, t, m), qT[:, t],
                                 start=(t == 0), stop=(t == 3))
            nc.scalar.activation(
                dst[:, m], ps[:, 0:BL * P].rearrange("p (b q) -> p b q", b=BL),
                ACTF.Identity, bias=bt[:, m:m + 1])

    GS = []
    for _b in range(BL):
        gs_t = gsp.tile([P, H, NCH, P], F32, tag=f"gs{_b}")
        GS.append(gs_t)

    # ================= per-sample =================
    for b in range(BL):
        # ---- keyT ----
        keyT = []
        for _t in range(4):
            kt_t = ktpool.tile([P, KP], F32R, tag=f"kt{_t}")
            keyT.append(kt_t)
        for c in range(NCH):
            kch = smallp.tile([P, D], F32R, tag="ld")
            rows = K - c * P if c == NCH - 1 else P
            dma.dma_start(kch[0:rows], key_d[b, c * P:c * P + rows])
            for t in range(4):
                ps = ps_tr.tile([P, P], F32R, tag="tr")
                nc.tensor.transpose(ps, kch[:, t * P:(t + 1) * P], ident_r)
                nc.vector.tensor_copy(keyT[t][:, c * P:(c + 1) * P], ps)

        # ---- kcT + chunk energies + sexp / rden (ln-exp table) ----
        wkc = load_w("Wk_c")
        kcT = []
        for _t in range(4):
            kc_t = kcpool.tile([P, KP], F32R, tag=f"kc{_t}")
            kcT.append(kc_t)
        for m in range(4):
            for n in range(3):
                o = n * 512
                ps = ps_mid.tile([P, 512], F32, tag="mid")
                for t in range(4):
                    nc.tensor.matmul(ps[:, 0:NB3P[n]], wsl(wkc, t, m),
                                     keyT[t][:, o:o + NB3P[n]],
                                     start=(t == 0), stop=(t == 3))
                nc.scalar.activation(kcT[m][:, o:o + NB3P[n]], ps[:, 0:NB3P[n]],
                                     ACTF.Identity, bias=bkc_t[:, m:m + 1])

        def ec_mm(n):
            o = n * 512
            ps = ps_mid.tile([P, 512], F32, tag="mid")
            for t in range(4):
                nc.tensor.matmul(ps[:, 0:NB3[n]], qcT[:, t, b],
                                 kcT[t][:, o:o + NB3[n]],
                                 start=(t == 0), stop=(t == 3))
            return ps

        nmx = smallp.tile([P, 4], F32, tag="nmx")
        for n in range(3):
            ps = ec_mm(n)
            nc.vector.tensor_reduce(nmx[:, n:n + 1], ps[:, 0:NB3[n]], AX.X,
                                    op=ALU.max, negate=True)
        nc.vector.tensor_reduce(nmx[:, 3:4], nmx[:, 0:3], AX.X, op=ALU.min)
        nc.vector.tensor_scalar_mul(nmx[:, 3:4], nmx[:, 3:4], INV_S)
        sexp = pairp.tile([P, KP], F32, tag="t1")
        nc.vector.memset(sexp[:, K:KP], 0.0)
        for n in range(3):
            ps = ec_mm(n)
            o = n * 512
            nc.scalar.activation(sexp[:, o:o + NB3[n]], ps[:, 0:NB3[n]],
                                 ACTF.Exp, bias=nmx[:, 3:4], scale=INV_S)
        nc.gpsimd.tensor_scalar_max(sexp[:, 0:K], sexp[:, 0:K], 1e-5)

        csd = pairp.tile([P, 9 + KP], F32, tag="t2")
        nc.vector.memset(csd[:, 0:9], 0.0)
        nc.vector.tensor_tensor_scan(csd[:, 9:9 + KP], zeros_w, sexp, 0.0,
                                     op0=ALU.add, op1=ALU.add)
        den = pairp.tile([P, KP], F32, tag="t3")
        nc.vector.tensor_sub(den[:, 0:K], csd[:, 9:9 + K], csd[:, 1:1 + K])
        rden = pairp.tile([P, KP], F32, tag="t4")
        nc.vector.memset(rden[:, K:KP], 0.0)
        nc.vector.reciprocal(rden[:, 0:K], den[:, 0:K])

        rdenT = rdtp.tile([P, NCH, P], F32, tag="rdenT")
        for c in range(NCH):
            ps = ps_tr.tile([P, P], F32, tag="tr")
            nc.tensor.transpose(ps, rden[:, c * P:(c + 1) * P], ident)
            nc.scalar.activation(rdenT[:, c], ps, ACTF.Copy)
        for c in range(NCH):
            ps = ps_tr.tile([P, P], F32, tag="tr")
            nc.tensor.transpose(ps, sexp[:, c * P:(c + 1) * P], ident)
            sxs = smallp.tile([P, P], F32, tag="rcT")
            nc.scalar.activation(sxs, ps, ACTF.Copy)
            dma.dma_start(sexpT_d[b, c], sxs)

        # ---- per-head: kmT, monotonic path, G / u ----
        wkm = load_w("Wk_m")
        for h in range(4):
            pr = b * H + h
            kmT = kcpool.tile([P, KP], F32R, tag="kc0")
            for n in range(3):
                o = n * 512
                ps = ps_mid.tile([P, 512], F32, tag="mid")
                for t in range(4):
                    nc.tensor.matmul(ps[:, 0:NB3P[n]], wsl(wkm, t, h),
                                     keyT[t][:, o:o + NB3P[n]],
                                     start=(t == 0), stop=(t == 3))
                nc.scalar.activation(kmT[:, o:o + NB3P[n]], ps[:, 0:NB3P[n]],
                                     ACTF.Identity, bias=bkm_t[:, h:h + 1])
            q1mp = pairp.tile([P, KP], F32, tag="t1")
            for n in range(3):
                o = n * 512
                ps = ps_mid.tile([P, 512], F32, tag="mid")
                nc.tensor.matmul(ps[:, 0:NB3[n]], qmT[:, h, b],
                                 kmT[:, o:o + NB3[n]], start=True, stop=True)
                nc.scalar.activation(q1mp[:, o:o + NB3[n]], ps[:, 0:NB3[n]],
                                     ACTF.Sigmoid, bias=neg_r, scale=-INV_S)
            nc.gpsimd.tensor_scalar_max(q1mp[:, 0:K], q1mp[:, 0:K], 1e-6)
            lg = pairp.tile([P, KP], F32, tag="t3")
            nc.scalar.activation(lg[:, 0:K], q1mp[:, 0:K], ACTF.Ln)
            p_t = pairp.tile([P, KP], F32, tag="t4")
            nc.scalar.activation(p_t[:, 0:K], q1mp[:, 0:K], ACTF.Identity,
                                 bias=1.0, scale=-1.0)
            csb = pairp.tile([P, 9 + KP], F32, tag="t2")
            nc.vector.memset(csb[:, 0:1], 0.0)
            nc.vector.tensor_tensor_scan(csb[:, 1:1 + K], zeros_w[:, 0:K],
                                         lg[:, 0:K], 0.0, op0=ALU.add,
                                         op1=ALU.add)
            cp = pairp.tile([P, KP], F32, tag="t1")
            nc.scalar.activation(cp[:, 0:K], csb[:, 0:K], ACTF.Exp)
            pc = pairp.tile([P, KP], F32, tag="pc")
            nc.vector.memset(pc[:, K:KP], 0.0)
            nc.gpsimd.tensor_mul(pc[:, 0:K], p_t[:, 0:K], cp[:, 0:K])
            nc.gpsimd.tensor_scalar_max(cp[:, 0:K], cp[:, 0:K], 1e-6)
            rcp = pairp.tile([P, KP], F32, tag="rcp")
            nc.vector.memset(rcp[:, K:KP], 0.0)
            nc.vector.reciprocal(rcp[:, 0:K], cp[:, 0:K])
            for c in range(NCH):
                ps_pc = ps_ms.tile([P, P], F32, tag="mscv")
                nc.tensor.transpose(ps_pc, pc[:, c * P:(c + 1) * P], ident)
                ps_rc = ps_tr.tile([P, P], F32, tag="tr")
                nc.tensor.transpose(ps_rc, rcp[:, c * P:(c + 1) * P], ident)
                rcT = smallp.tile([P, P], F32, tag="rcT")
                nc.scalar.activation(rcT, ps_rc, ACTF.Copy)
                nc.vector.tensor_mul(GS[b][:, h, c, 1:P], ps_pc[:, 0:P - 1],
                                     rcT[:, 1:P])
                uT = smallp.tile([P, P], F32, tag="uT")
                nc.vector.tensor_mul(uT, ps_pc, rdenT[:, c])
                dma.dma_start(u_d[pr, c], uT)

        # ---- valueT + v ----
        wv = load_w("Wv")
        valT = []
        for _t in range(4):
            vt_t = ktpool.tile([P, KP], F32R, tag=f"kt{_t}")
            valT.append(vt_t)
        for c in range(NCH):
            vch = smallp.tile([P, D], F32R, tag="ld")
            rows = K - c * P if c == NCH - 1 else P
            dma.dma_start(vch[0:rows], value_d[b, c * P:c * P + rows])
            for t in range(4):
                ps = ps_tr.tile([P, P], F32R, tag="tr")
                nc.tensor.transpose(ps, vch[:, t * P:(t + 1) * P], ident_r)
                nc.vector.tensor_copy(valT[t][:, c * P:(c + 1) * P], ps)
        for c in range(NCH):
            ps = ps_mid.tile([P, 512], F32, tag="mid")
            for t in range(4):
                nc.tensor.matmul(ps, valT[t][:, c * P:(c + 1) * P], wv[:, t],
                                 start=(t == 0), stop=False)
            nc.tensor.matmul(ps, ones_r[0:1, 0:P], bv_row, start=False, stop=True)
            vst = smallp.tile([P, D], F32, tag="ld")
            if c == NCH - 1:
                nc.vector.memset(vst, 0.0)
                nc.scalar.activation(vst[0:K - c * P], ps[0:K - c * P], ACTF.Copy)
            else:
                nc.scalar.activation(vst, ps, ACTF.Copy)
            dma.dma_start(v_d[b, c], vst)

        # ================= phase B: diagonal wavefront =================
        gs = GS[b]
        gsf = gs.rearrange("p h c q -> p (h c q)")
        nc.vector.memset(gs[:, :, :, 0:1], 1.0)

        def diag_ap(dd, clo, cnt):
            return bass.AP(tensor=gsf.tensor,
                           offset=gsf.offset + dd + 127 * clo,
                           ap=[list(gsf.ap[0]), [127, cnt], [NCH * P, H]])

        for d in range(1, Q + NCH - 1):
            cmin = max(0, d - (Q - 1))
            cmax = min(NCH - 1, d - 1)
            ncl = cmax - cmin + 1
            c1 = max(cmin, 1)
            n2 = cmax - c1 + 1
            z = zpool.tile([P, NCH, H], F32, tag="z")
            nc.vector.tensor_mul(z[:, 0:ncl], diag_ap(d, cmin, ncl),
                                 diag_ap(d - 1, cmin, ncl))
            ps = ps_sc.tile([P, NCH, H], F32, tag="sc")
            nc.tensor.matmul(ps[:, 0:ncl].rearrange("p a b -> p (a b)"), L,
                             z[:, 0:ncl].rearrange("p a b -> p (a b)"),
                             start=True, stop=(n2 == 0))
            if n2 > 0:
                nc.tensor.matmul(
                    ps[:, c1 - cmin:c1 - cmin + n2].rearrange("p a b -> p (a b)"),
                    M127, diag_ap(d - 128, c1, n2), start=False, stop=True)
            nc.scalar.activation(diag_ap(d, cmin, ncl), ps[:, 0:ncl],
                                 ACTF.Copy)

        # ================= phase C: beta + context =================
        sxT = [None, None]
        vtl = [None, None]
        t_prev = [None] * 4
        t_cur = [None] * 4
        cv_sb = cpool.tile([P, H, P], F32, tag="cvsb")

        def emit_chunk(cc, tp, tn):
            par = cc % 2
            for h in range(4):
                ms = ps_ms.tile([P, P], F32, tag="mscv")
                nc.tensor.matmul(ms, Band0, tp[h], start=True, stop=(tn is None))
                if tn is not None:
                    nc.tensor.matmul(ms, Band1, tn[h], start=False, stop=True)
                bT = cpool.tile([P, P], F32, tag="bT")
                nc.vector.tensor_mul(bT, sxT[par], ms)
                cvp = ps_ms.tile([P, P], F32, tag="mscv")
                nc.tensor.matmul(cvp, vtl[par][:, h * P:(h + 1) * P], bT,
                                 start=True, stop=True)
                if cc == 0:
                    nc.vector.tensor_copy(cv_sb[:, h], cvp)
                else:
                    nc.vector.tensor_add(cv_sb[:, h], cv_sb[:, h], cvp)

        for c in range(NCH):
            par = c % 2
            sxT_t = cpool.tile([P, P], F32, tag="sxT")
            sxT[par] = sxT_t
            dma.dma_start(sxT_t, sexpT_d[b, c])
            vtl_t = cpool.tile([P, D], F32, tag="vtl")
            vtl[par] = vtl_t
            dma.dma_start(vtl_t, v_d[b, c])
            for h in range(4):
                uT = cpool.tile([P, P], F32, tag="uTl")
                dma.dma_start(uT, u_d[b * H + h, c])
                t_c = tpool.tile([P, P], F32, tag="t_c")
                nc.gpsimd.tensor_mul(t_c, gs[:, h, c, :], uT)
                t_cur[h] = t_c
            if c > 0:
                emit_chunk(c - 1, t_prev, t_cur)
            t_prev = list(t_cur)
        emit_chunk(NCH - 1, t_prev, None)

        # ---- output projection ----
        cvT = cpool.tile([P, H, P], F32R, tag="vtl")
        nc.scalar.activation(cvT, cv_sb, ACTF.Copy)
        wout = load_w("Wout")
        ops = ps_mid.tile([P, 512], F32, tag="mid")
        for h in range(4):
            nc.tensor.matmul(ops, cvT[:, h], wout[:, h],
                             start=(h == 0), stop=False)
        nc.tensor.matmul(ops, ones_r[0:1, 0:P], bout_row, start=False, stop=True)
        ost = smallp.tile([P, D], F32, tag="ld")
        nc.scalar.activation(ost, ops, ACTF.Copy)
        dma.dma_start(out_d[b], ost)


def build_nc():
    if "nc" in _CACHE:
        return _CACHE["nc"]
    nc = bacc.Bacc("TRN2", target_bir_lowering=False, debug=False)
    io = {}
    io["key2"] = nc.dram_tensor("key2", [BL, K, D], F32R, kind="ExternalInput")
    io["query2"] = nc.dram_tensor("query2", [BL, Q, D], F32R, kind="ExternalInput")
    io["value2"] = nc.dram_tensor("value2", [BL, K, D], F32R, kind="ExternalInput")
    for w in ("Wk_m", "Wq_m", "Wk_c", "Wq_c", "Wv", "Wout"):
        io[w] = nc.dram_tensor(w, [D, ADIM], F32R, kind="ExternalInput")
    for bn in ("bk_m", "bq_m", "bk_c", "bq_c"):
        io[bn] = nc.dram_tensor(bn, [ADIM], F32, kind="ExternalInput")
    io["bv"] = nc.dram_tensor("bv", [ADIM], F32R, kind="ExternalInput")
    io["bout"] = nc.dram_tensor("bout", [D], F32R, kind="ExternalInput")
    io["r"] = nc.dram_tensor("r", [1], F32, kind="ExternalInput")
    io["out2"] = nc.dram_tensor("out2", [BL, Q, D], F32, kind="ExternalOutput")
    io["u_sc"] = nc.dram_tensor("u_sc", [NPAIR, NCH, P, P], F32, kind="Internal")
    io["v_sc"] = nc.dram_tensor("v_sc", [BL, NCH, P, D], F32, kind="Internal")
    io["sexpT_sc"] = nc.dram_tensor("sexpT_sc", [BL, NCH, P, P], F32,
                                    kind="Internal")

    with tile.TileContext(nc) as tc, ExitStack() as ctx:
        _emit(nc, tc, ctx, io)
    nc.compile()
    _CACHE["nc"] = nc
    return nc


def make_in_maps(inputs):
    f = lambda x: np.ascontiguousarray(np.asarray(x), dtype=np.float32)
    shared = {w: f(inputs[w]) for w in
              ("Wk_m", "Wq_m", "Wk_c", "Wq_c", "Wv", "Wout",
               "bk_m", "bq_m", "bk_c", "bq_c", "bv", "bout")}
    shared["r"] = f(inputs["r"]).reshape(1)
    key, query, value = f(inputs["key"]), f(inputs["query"]), f(inputs["value"])
    in_maps = []
    for i in range(NCORE):
        s = slice(i * BL, (i + 1) * BL)
        in_maps.append(dict(shared, key2=key[s], query2=query[s],
                            value2=value[s]))
    return in_maps


def kernel(**inputs):
    nc = build_nc()
    in_maps = make_in_maps(inputs)
    res = run_bass_kernel_spmd(nc, in_maps, core_ids=list(range(NCORE)))
    out = np.concatenate([r["out2"] for r in res.results], axis=0)
    return out.astype(np.float32)
